# revision 4
# baseline (speedup 1.0000x reference)
"""Trainium2 Bass kernel for nn_CustomConv2d: 3x3 conv, B=16, Cin=Cout=128, H=W=64.

Strategy (v2):
  - Data-parallel over batch: 8 NeuronCores x 2 images each; the (128,128,9)
    weight is replicated (host pre-transposes to [cin, tap, cout] so tap k is
    a contiguous [cin, cout] stationary-operand slice).
  - All matmul operands are bf16 (same 1 cycle/row PE rate as fp32r, half the
    DMA bytes; quantization error ~1e-3 rel, far under the 2e-2 gate).
    Host converts x and w to bf16; PSUM accumulates fp32; output is fp32.
  - Per image the feature map lives in SBUF as a 66x66 zero-padded plane
    (host-prepadded, so every DMA is fully contiguous per partition).
  - Conv = 9 accumulating PE matmuls per 8-row output block (contraction over
    Cin=128 on the partition dim); tap (dy,dx) reads the 2D window
    [[66,8],[1,64]] at offset (y0+dy)*66 + dx.
  - DMA plan exploits the ~1.8us fixed latency chain per DMA instruction
    (seq config + HWDGE gen + DGE->DMA delay) and the 0.9us completion
    semaphore: few instructions, spread across engines so sequencer config
    runs in parallel, first chunks small so the first matmul's data lands
    ASAP: x img0 on vector (rows 0-9 first), w on scalar (taps 0-2 first),
    x img1 on sync, per-block output DMAs on gpsimd (SWDGE: 25ns sequencer
    cost, separate software descriptor generator).
  - PE warm-up: HAM un-throttles the PE clock 1.2->2.4 GHz only after ~3us
    of sustained array activity; bf16 dummy matmuls on a zeroed tile bridge
    from engine start (~6us) to first-data-ready (~9.5us) so conv matmuls
    run at full clock from the first block with no idle gap.
  - Tail: the last block gets a single copy + single DMA on the idle sync
    engine (minimum fixed-latency chain after the final matmul).
"""

import numpy as np
import ml_dtypes

import concourse.bass as bass  # noqa: F401  (registers bass types)
import concourse.tile as tile
import concourse.mybir as mybir
from concourse import bacc, bass_utils

F32 = mybir.dt.float32
BF16 = mybir.dt.bfloat16

B, CIN, COUT, KK, H, W = 16, 128, 128, 3, 64, 64
NCORES = 8
BPC = B // NCORES  # images per core
HW = H * W         # 4096
PW = W + 2         # padded row length (66)
PH = H + 2         # padded rows (66)
XLEN = PH * PW     # 4356
ROWBLK = 8         # output rows per PSUM block (8*64=512 = one fp32 PSUM bank)
NBLK = H // ROWBLK # 8 blocks per image

WARMN = 10         # warmup matmuls (bridge engine-start -> first data ready)
TRACE = False      # set True to capture an NTFF profile (fills LAST_EXEC_NS)
LAST_EXEC_NS = None

_CACHE = {}

# x chunk row ranges per image (padded-row indices).  Block yb consumes
# padded rows [8*yb, 8*yb+10).  First chunk minimal so block 0 starts ASAP.
CHUNKS0 = [(0, 10), (10, 34), (34, 58), (58, PH)]   # img0, on vector
CHUNKS1 = [(0, 22), (22, 44), (44, PH)]             # img1, on sync


def _build():
    nc = bacc.Bacc("TRN2", target_bir_lowering=False, debug=False, num_devices=NCORES)
    x_d = nc.dram_tensor("x", [BPC, CIN, XLEN], BF16, kind="ExternalInput").ap()
    w_d = nc.dram_tensor("w", [CIN, KK * KK * COUT], BF16, kind="ExternalInput").ap()
    o_d = nc.dram_tensor("o", [BPC, COUT, HW], F32, kind="ExternalOutput").ap()

    with tile.TileContext(nc) as tc:
        with (
            tc.tile_pool(name="wt", bufs=1) as wtp,
            tc.tile_pool(name="xin", bufs=2) as xp,
            tc.tile_pool(name="ps", bufs=6, space="PSUM") as pp,
            tc.tile_pool(name="ot", bufs=4) as op,
            tc.tile_pool(name="warm", bufs=1) as wmp,
            tc.tile_pool(name="warmps", bufs=1, space="PSUM") as wpp,
        ):
            # --- warmup: keep PE busy from engine start until data lands ---
            wz = wmp.tile([CIN, 4 * COUT], BF16)
            nc.gpsimd.memset(wz[:], 0.0)
            wps = wpp.tile([COUT, 4 * COUT], F32)
            for _ in range(WARMN):
                nc.tensor.matmul(wps[:], wz[:, :COUT], wz[:], start=True, stop=True)

            # --- input DMAs, spread across engines ---
            wt = wtp.tile([CIN, KK * KK * COUT], BF16)
            xins = []
            for lb in range(BPC):
                xin = xp.tile([CIN, XLEN], BF16, tag="xin")
                xins.append(xin)

            # img0 chunk 0 (rows 0-9) on sync: the first-matmul dependency
            r0, r1 = CHUNKS0[0]
            nc.sync.dma_start(xins[0][:, PW * r0 : PW * r1], x_d[0][:, PW * r0 : PW * r1])
            # w taps 0-2 then 3-8 on scalar
            nc.scalar.dma_start(wt[:, : 3 * COUT], w_d[:, : 3 * COUT])
            nc.scalar.dma_start(wt[:, 3 * COUT :], w_d[:, 3 * COUT :])
            # img0 remaining chunks on sync
            for r0, r1 in CHUNKS0[1:]:
                nc.sync.dma_start(
                    xins[0][:, PW * r0 : PW * r1], x_d[0][:, PW * r0 : PW * r1]
                )
            # img1 on scalar (early, idle after the w loads)
            for r0, r1 in CHUNKS1:
                nc.scalar.dma_start(
                    xins[1][:, PW * r0 : PW * r1], x_d[1][:, PW * r0 : PW * r1]
                )

            # --- conv: 9 accumulating matmuls per 8-row block ---
            for lb in range(BPC):
                xrf = xins[lb][:].rearrange("p (r c) -> p r c", c=PW)  # [128,66,66]
                for yb in range(NBLK):
                    y0 = yb * ROWBLK
                    ps = pp.tile([COUT, ROWBLK * W], F32)
                    first = True
                    for dy in range(KK):
                        for dx in range(KK):
                            nc.tensor.matmul(
                                ps[:],
                                wt[:, (dy * KK + dx) * COUT : (dy * KK + dx + 1) * COUT],
                                xrf[:, y0 + dy : y0 + dy + ROWBLK, dx : dx + W],
                                start=first,
                                stop=(dy == KK - 1 and dx == KK - 1),
                            )
                            first = False
                    ot = op.tile([COUT, ROWBLK * W], F32)
                    nc.vector.tensor_copy(ot[:], ps[:])
                    if lb == BPC - 1 and yb == NBLK - 1:
                        # final block: minimum-latency chain on the idle sync engine
                        nc.sync.dma_start(
                            o_d[lb][:, W * y0 : W * y0 + ROWBLK * W], ot[:]
                        )
                    else:
                        nc.gpsimd.dma_start(
                            o_d[lb][:, W * y0 : W * y0 + ROWBLK * W], ot[:]
                        )
    nc.compile()
    return nc


def _get_nc():
    key = "nc_v2"
    if key not in _CACHE:
        _CACHE[key] = _build()
    return _CACHE[key]


def kernel(x, weights):
    """x: [16,128,64,64] f32; weights: [128,128,9] f32 -> [2048,64,64] f32."""
    global LAST_EXEC_NS
    x = np.asarray(x, dtype=np.float32)
    w = np.asarray(weights, dtype=np.float32)
    # [cout, cin, k] -> [cin, k, cout] so tap k is a contiguous lhsT slice
    wT = np.ascontiguousarray(w.transpose(1, 2, 0)).reshape(CIN, KK * KK * COUT)
    wT = wT.astype(ml_dtypes.bfloat16)
    xpad = np.zeros((B, CIN, PH, PW), ml_dtypes.bfloat16)
    xpad[:, :, 1 : H + 1, 1 : W + 1] = x.astype(ml_dtypes.bfloat16)
    xpad = xpad.reshape(B, CIN, XLEN)

    nc = _get_nc()
    xr = xpad.reshape(NCORES, BPC, CIN, XLEN)
    in_maps = [{"x": np.ascontiguousarray(xr[c]), "w": wT} for c in range(NCORES)]

    res = bass_utils.run_bass_kernel_spmd(
        nc, in_maps, core_ids=list(range(NCORES)), trace=TRACE
    )
    LAST_EXEC_NS = res.exec_time_ns

    arr = np.stack([res.results[c]["o"] for c in range(NCORES)])  # [8, 2, 128, 4096]
    # out[cout*B + b] = conv[b, cout], with b = core*BPC + lb
    arr = arr.transpose(2, 0, 1, 3).reshape(COUT, B, H, W)
    return np.ascontiguousarray(arr.reshape(COUT * B, H, W))


# revision 5
# speedup vs baseline: 1.0548x; 1.0548x over previous
"""Trainium2 Bass kernel for nn_CustomConv2d: 3x3 conv, B=16, Cin=Cout=128, H=W=64.

Strategy (v3):
  - Data-parallel over batch: 8 NeuronCores x 2 images each; the (128,128,9)
    weight is replicated (host pre-transposes to [cin, tap, cout] so tap k is
    a contiguous [cin, cout] stationary-operand slice).
  - fp32r matmuls (TF32-like, 1 cycle/row; bf16 moving operands measured
    SLOWER, ~250ns vs ~231ns per 512-row matmul).  Host pre-rounds inputs to
    fp32r so on-device numerics are deterministic.
  - Per image the feature map lives in SBUF as a 66x66 zero-padded plane
    (host-prepadded => every DMA is contiguous per partition).
  - Conv = 9 accumulating PE matmuls per 8-row output block (contraction over
    Cin=128 on the partition dim).
  - DMA plan built around the measured fixed costs per DMA instruction
    (~0.6us sequencer config + ~0.63us on the GLOBAL HWDGE generator +
    ~0.65us DGE->ring delay + 0.9us completion-semaphore propagation):
    few instructions, first chunks minimal, split across the two HWDGE
    engines (sync=SP carries img0 x chunks; scalar=Activation carries w in
    3 tap-groups then img1 chunks then per-block output DMAs).
  - PE warm-up: HAM un-throttles the PE clock only after ~2.5-3.4us of
    sustained array activity AND re-throttles after an idle window, so the
    warmup (bf16 junk matmuls on a zeroed tile, memset on the otherwise-idle
    vector engine) is sized to end exactly when the first conv data lands.
  - Tail: final block copied in halves with the two DMAs on sync+scalar so
    the kernel-exit drain starts as soon as possible.
"""

import numpy as np

import concourse.bass as bass  # noqa: F401  (registers bass types)
import concourse.tile as tile
import concourse.mybir as mybir
from concourse import bacc, bass_utils

F32 = mybir.dt.float32
F32R = mybir.dt.float32r
BF16 = mybir.dt.bfloat16

B, CIN, COUT, KK, H, W = 16, 128, 128, 3, 64, 64
NCORES = 8
BPC = B // NCORES  # images per core
HW = H * W         # 4096
PW = W + 2         # padded row length (66)
PH = H + 2         # padded rows (66)
XLEN = PH * PW     # 4356
ROWBLK = 8         # output rows per PSUM block (8*64=512 = one fp32 PSUM bank)
NBLK = H // ROWBLK # 8 blocks per image

WARMN = 8          # warmup matmuls (bridge engine-start -> first data ready)
BF16_W = False     # stationary (weights) in bf16, moving in f32r (cadence test)
TRACE = False      # set True to capture an NTFF profile (fills LAST_EXEC_NS)
LAST_EXEC_NS = None

_CACHE = {}

# x chunk row ranges (padded-row indices).  Block yb consumes rows [8yb, 8yb+10).
CHUNKS0 = [(0, 10), (10, 22), (22, 46), (46, PH)]   # img0, on sync
CHUNKS1 = [(0, 22), (22, 44), (44, PH)]             # img1, on scalar


def _build():
    wdt = BF16 if BF16_W else F32R
    nc = bacc.Bacc("TRN2", target_bir_lowering=False, debug=False, num_devices=NCORES)
    x_d = nc.dram_tensor("x", [BPC, CIN, XLEN], F32R, kind="ExternalInput").ap()
    w_d = nc.dram_tensor("w", [CIN, KK * KK * COUT], wdt, kind="ExternalInput").ap()
    o_d = nc.dram_tensor("o", [BPC, COUT, HW], F32, kind="ExternalOutput").ap()

    with tile.TileContext(nc) as tc:
        with (
            tc.tile_pool(name="wt", bufs=1) as wtp,
            tc.tile_pool(name="xin", bufs=2) as xp,
            tc.tile_pool(name="ps", bufs=6, space="PSUM") as pp,
            tc.tile_pool(name="ot", bufs=4) as op,
            tc.tile_pool(name="warm", bufs=1) as wmp,
            tc.tile_pool(name="warmps", bufs=1, space="PSUM") as wpp,
        ):
            # --- warmup: keep the PE busy from engine start until data lands.
            # memset on vector (earliest-starting engine, idle until the first
            # PSUM eviction at ~12us).
            wz = wmp.tile([CIN, 4 * COUT], BF16)
            nc.vector.memset(wz[:], 0.0)
            wps = wpp.tile([COUT, 4 * COUT], F32)
            for _ in range(WARMN):
                nc.tensor.matmul(wps[:], wz[:, :COUT], wz[:], start=True, stop=True)

            # --- input DMAs ---
            wt = wtp.tile([CIN, KK * KK * COUT], wdt)
            xins = []
            for lb in range(BPC):
                xin = xp.tile([CIN, XLEN], F32R, tag="xin")
                xins.append(xin)

            # img0 chunk 0 (rows 0-9) on sync: first-matmul dependency,
            # biggest transfer of the critical pair -> first in HWDGE order.
            r0, r1 = CHUNKS0[0]
            nc.sync.dma_start(xins[0][:, PW * r0 : PW * r1], x_d[0][:, PW * r0 : PW * r1])
            # w in 3 tap-groups on scalar (tap k is consumed at ~t0+0.24k us)
            for g in range(3):
                nc.scalar.dma_start(
                    wt[:, g * 3 * COUT : (g + 1) * 3 * COUT],
                    w_d[:, g * 3 * COUT : (g + 1) * 3 * COUT],
                )
            # img0 remaining chunks on sync
            for r0, r1 in CHUNKS0[1:]:
                nc.sync.dma_start(
                    xins[0][:, PW * r0 : PW * r1], x_d[0][:, PW * r0 : PW * r1]
                )
            # img1 on scalar (after w; needed only from ~27us)
            for r0, r1 in CHUNKS1:
                nc.scalar.dma_start(
                    xins[1][:, PW * r0 : PW * r1], x_d[1][:, PW * r0 : PW * r1]
                )

            # --- conv: 9 accumulating matmuls per 8-row block ---
            for lb in range(BPC):
                xrf = xins[lb][:].rearrange("p (r c) -> p r c", c=PW)  # [128,66,66]
                for yb in range(NBLK):
                    y0 = yb * ROWBLK
                    ps = pp.tile([COUT, ROWBLK * W], F32)
                    first = True
                    for dy in range(KK):
                        for dx in range(KK):
                            nc.tensor.matmul(
                                ps[:],
                                wt[:, (dy * KK + dx) * COUT : (dy * KK + dx + 1) * COUT],
                                xrf[:, y0 + dy : y0 + dy + ROWBLK, dx : dx + W],
                                start=first,
                                stop=(dy == KK - 1 and dx == KK - 1),
                            )
                            first = False
                    ot = op.tile([COUT, ROWBLK * W], F32)
                    if lb == BPC - 1 and yb == NBLK - 1:
                        # final block in halves so copy/store pipeline and the
                        # kernel-exit drain starts sooner
                        hw2 = ROWBLK * W // 2
                        for h_, eng in ((0, nc.sync), (1, nc.scalar)):
                            sl = slice(h_ * hw2, (h_ + 1) * hw2)
                            nc.vector.tensor_copy(ot[:, sl], ps[:, sl])
                            eng.dma_start(
                                o_d[lb][:, W * y0 + h_ * hw2 : W * y0 + (h_ + 1) * hw2],
                                ot[:, sl],
                            )
                    else:
                        nc.vector.tensor_copy(ot[:], ps[:])
                        nc.scalar.dma_start(
                            o_d[lb][:, W * y0 : W * y0 + ROWBLK * W], ot[:]
                        )
    nc.compile()
    return nc


def _get_nc():
    key = ("nc_v3", BF16_W, WARMN)
    if key not in _CACHE:
        _CACHE[key] = _build()
    return _CACHE[key]


def _round_f32r(a):
    """RNE-round fp32 values to fp32r (keep top 20 bits: 1s+8e+11m)."""
    u = np.ascontiguousarray(a, dtype=np.float32).view(np.uint32)
    lsb = (u >> np.uint32(12)) & np.uint32(1)
    r = u + np.uint32(0x7FF) + lsb
    return (r & np.uint32(0xFFFFF000)).view(np.float32)


def kernel(x, weights):
    """x: [16,128,64,64] f32; weights: [128,128,9] f32 -> [2048,64,64] f32."""
    global LAST_EXEC_NS
    x = np.asarray(x, dtype=np.float32)
    w = np.asarray(weights, dtype=np.float32)
    # [cout, cin, k] -> [cin, k, cout] so tap k is a contiguous lhsT slice
    wT = np.ascontiguousarray(w.transpose(1, 2, 0)).reshape(CIN, KK * KK * COUT)
    if BF16_W:
        import ml_dtypes

        wT = wT.astype(ml_dtypes.bfloat16)
    else:
        wT = _round_f32r(wT)
    xpad = np.zeros((B, CIN, PH, PW), np.float32)
    xpad[:, :, 1 : H + 1, 1 : W + 1] = x
    xpad = _round_f32r(xpad.reshape(-1)).reshape(B, CIN, XLEN)

    nc = _get_nc()
    xr = xpad.reshape(NCORES, BPC, CIN, XLEN)
    in_maps = [{"x": np.ascontiguousarray(xr[c]), "w": wT} for c in range(NCORES)]

    res = bass_utils.run_bass_kernel_spmd(
        nc, in_maps, core_ids=list(range(NCORES)), trace=TRACE
    )
    LAST_EXEC_NS = res.exec_time_ns

    arr = np.stack([res.results[c]["o"] for c in range(NCORES)])  # [8, 2, 128, 4096]
    # out[cout*B + b] = conv[b, cout], with b = core*BPC + lb
    arr = arr.transpose(2, 0, 1, 3).reshape(COUT, B, H, W)
    return np.ascontiguousarray(arr.reshape(COUT * B, H, W))
